# revision 39
# baseline (speedup 1.0000x reference)
"""BiMamba forward kernel for 8 TRN2 NeuronCores.

Sharding: core c = (batch b, direction dir, d_inner half h); the host
pre-flips reverse-direction inputs in time so the device program is
identical (purely causal) on all cores. Each core produces two partial
output projections [d_model, L] (r-tiles 0..4 and r-tile 5); the host
sums them and the four core-partials per batch element (unflipping the
reverse ones). A host-side channel permutation puts this core's d_inner
half in x-path tiles 0..5 so the single SPMD program needs no per-core
branches.

Device layout: channels on partitions, time on the free dim. The scan
is hardware tensor_tensor_scan (h = dA*h + dBu along time). All scan-
phase elementwise work runs on VectorE only (GpSimd shares an SBUF port
with the DVE, so offloading there halves DVE throughput); everything is
bf16 SBUF step-1 to hit the DVE 2x perf mode. States are processed in
fused pairs: one [128, 2L] tile per quantity, with dA's column L pinned
to zero so the concatenated scan resets exactly at the pair boundary
(h_0 never reads dA_0). delta = softplus(dt@W_dt+b_dt) is computed
entirely on ScalarE as Ln(1 + Exp(zb)) -- zb is empirically in [-8, 2]
so the naive form is safe -- with ACT calls batched per function to
minimize activation-table reloads. x_dbl partials are pairwise
AllReduced (replica groups [2p, 2p+1]), split into dt rows and B/C rows
so the dt rows that gate the delta->dA chain land first. The Dp-skip
and sum-over-states accumulate in PSUM via identity/diagonal matmuls on
TensorE; the causal conv runs on TensorE as 4 diagonal matmuls over
shifted views. Out-proj pass A (r 0..4) is drip-fed into the PE queue
while the last r-tile's scan still occupies VectorE; pass B (r=5, bf16)
is the only post-scan work and runs in a deep-buffered
matmul/copy/DMA pipeline.
"""
import numpy as np
import ml_dtypes

import concourse.bass as bass
import concourse.tile as tile
from concourse import bacc, mybir
from concourse.bass_utils import run_bass_kernel_spmd

D_MODEL = 768
D_INNER = 1536
D_STATE = 16
D_CONV = 4
DT_RANK = 48
BATCH = 2
SEQLEN = 2048

HALF = D_INNER // 2
NDT = HALF // 128            # 6 half d-tiles
NDT_FULL = D_INNER // 128    # 12 full d-tiles
NK = D_MODEL // 128          # 6 k-tiles over d_model
L = SEQLEN
NCH = 4
CW = L // NCH                # 512
NXD = DT_RANK + 2 * D_STATE  # 80
NXP = 96                     # x_dbl rows: dt at 0..47, B/C at 64..95
NM = D_MODEL // 128          # 6 out-proj row tiles

F32 = mybir.dt.float32
BF16 = mybir.dt.bfloat16
BF_NP = ml_dtypes.bfloat16

N_S_F32 = 4                                         # fp32 decay planes
USE_CC = True    # pairwise x_dbl via pairwise AllReduce

AF = mybir.ActivationFunctionType
OP = mybir.AluOpType


def build_program(debug_stage=0):
    nc = bacc.Bacc("TRN2", target_bir_lowering=False, debug=False,
                   num_devices=8)
    dram = {}

    def din(name, shape, dt):
        dram[name] = nc.dram_tensor(name, list(shape), dt,
                                    kind="ExternalInput").ap()

    def dout(name, shape, dt):
        dram[name] = nc.dram_tensor(name, list(shape), dt,
                                    kind="ExternalOutput").ap()

    din("uT", (D_MODEL, L), BF16)
    din("w_in_xT", (D_MODEL, D_INNER), BF16)
    din("w_in_zT", (D_MODEL, HALF), BF16)
    din("conv_cols", (D_INNER, D_CONV), F32)
    din("conv_b", (D_INNER, 1), F32)
    din("w_xT", (D_INNER, NXP), BF16)
    din("w_dtT", (DT_RANK, HALF), BF16)
    din("b_dt", (HALF, 1), F32)
    din("A_half", (HALF, D_STATE), F32)
    din("dp_diag", (NDT * 128, 128), BF16)
    din("idn", (128, 128), BF16)
    din("w_outT", (HALF, D_MODEL), BF16)

    if debug_stage == 1:
        dout("xc_dbg", (HALF if USE_CC else D_INNER, L), F32)
        dout("delta_dbg", (HALF, L), F32)
        dout("xdbl_dbg", (NXP, L), F32)
    dout("out_part", (D_MODEL, L), F32)
    dout("out_part5", (D_MODEL, L), BF16)

    with tile.TileContext(nc) as tc:
        _body_once(nc, tc, dram, debug_stage)
    nc.compile()
    return nc


def _body_once(nc, tc, dram, dbg):
    with tc.tile_pool(name="wpool", bufs=1) as wp, \
         tc.tile_pool(name="dramp", bufs=1, space="DRAM") as dp_pool:

        # ---- DRAM scratch (tracked tiles) ----
        bc_scr = dp_pool.tile([2 * D_STATE, L], BF16, name="bc_scr")
        cc_in_dt = dp_pool.tile([DT_RANK, L], F32, name="cc_in_dt")
        cc_out_dt = dp_pool.tile([DT_RANK, L], F32, name="cc_out_dt")
        cc_in_bc = dp_pool.tile([2 * D_STATE, L], F32, name="cc_in_bc")
        cc_out_bc = dp_pool.tile([2 * D_STATE, L], F32, name="cc_out_bc")
        z_scr = [dp_pool.tile([128, L], BF16, name=f"z_scr{r}")
                 for r in range(NDT)]
        xc_scr = [dp_pool.tile([128, L], BF16, name=f"xc_scr{r}")
                  for r in range(NDT)]

        # ---- persistent small weights ----
        idn = wp.tile([128, 128], BF16, name="idn")
        nc.sync.dma_start(idn[:], dram["idn"][:])
        dp_diag = [wp.tile([128, 128], BF16, name=f"dpd{r}")
                   for r in range(NDT)]
        A_col = [wp.tile([128, D_STATE], F32, name=f"acol{r}")
                 for r in range(NDT)]
        b_dt = [wp.tile([128, 1], F32, name=f"bdt{r}") for r in range(NDT)]
        conv_b = [wp.tile([128, 1], F32, name=f"cvb{r}")
                  for r in range(NDT_FULL)]
        cw_cols = [wp.tile([128, D_CONV], F32, name=f"cwc{r}")
                   for r in range(NDT)]
        for r in range(NDT):
            nc.sync.dma_start(cw_cols[r][:],
                              dram["conv_cols"][r * 128:(r + 1) * 128, :])
        for r in range(NDT):
            nc.sync.dma_start(dp_diag[r][:],
                              dram["dp_diag"][r * 128:(r + 1) * 128, :])
            nc.sync.dma_start(A_col[r][:],
                              dram["A_half"][r * 128:(r + 1) * 128, :])
            nc.sync.dma_start(b_dt[r][:],
                              dram["b_dt"][r * 128:(r + 1) * 128, :])
        for r in range(NDT_FULL):
            nc.sync.dma_start(conv_b[r][:],
                              dram["conv_b"][r * 128:(r + 1) * 128, :])
        w_dtT = wp.tile([DT_RANK, HALF], BF16, name="w_dtT")
        nc.sync.dma_start(w_dtT[:], dram["w_dtT"][:])
        w_outT = [wp.tile([128, D_MODEL], BF16, name=f"wout{r}")
                  for r in range(NDT)]
        for r in range(NDT):
            nc.sync.dma_start(w_outT[r][:],
                              dram["w_outT"][r * 128:(r + 1) * 128, :])
        NDT_X = NDT if USE_CC else NDT_FULL
        w_xT = [wp.tile([128, NXP], BF16, name=f"wx{k}")
                for k in range(NDT_X)]
        for k in range(NDT_X):
            nc.sync.dma_start(w_xT[k][:],
                              dram["w_xT"][k * 128:(k + 1) * 128, :])

        with tc.tile_pool(name="hold", bufs=1) as hold:
            dtT_bf = hold.tile([DT_RANK, L], BF16, name="dtT_bf")
            yg_bf = [hold.tile([128, L], BF16, name=f"yg{r}")
                     for r in range(NDT)]

            _stages_123(nc, tc, dram, dbg, wp, locals())

            # ---------- stage 6: out-proj, two passes ----------
            # pass A (r=0..4) is drip-fed into the PE queue while the
            # r=5 scan still runs on VectorE; pass B (r=5 only) is the
            # only post-scan work. The host sums the two partials.
            with tc.tile_pool(name="op6", bufs=1) as p6, \
                 tc.tile_pool(name="ps6", bufs=2, space="PSUM") as ps6:
                def _mk_passA(m, n):
                    def go():
                        ps = ps6.tile([128, CW], F32, name="ps6t",
                                      tag="ps6t")
                        for r in range(NDT - 1):
                            nc.tensor.matmul(
                                ps[:], w_outT[r][:, m * 128:(m + 1) * 128],
                                yg_bf[r][:, n * CW:(n + 1) * CW],
                                start=(r == 0), stop=(r == NDT - 2))
                        ot = p6.tile([128, CW], F32, name="ot", tag="ot",
                                     bufs=2)
                        nc.scalar.copy(ot[:], ps[:])
                        nc.sync.dma_start(
                            dram["out_part"][m * 128:(m + 1) * 128,
                                             n * CW:(n + 1) * CW], ot[:])
                    return go
                passA = [_mk_passA(m, n)
                         for m in range(NM) for n in range(NCH)]
                _scan_stage(nc, tc, dram, dbg, wp, locals(), passA)

                # pass B: deep-buffered pipeline (scan PSUM pools are
                # closed by now), copies alternating scalar/vector
                with tc.tile_pool(name="op6b", bufs=1) as p6b, \
                     tc.tile_pool(name="ps6b", bufs=6,
                                  space="PSUM") as ps6b:
                    for i, (m, n) in enumerate(
                            (m, n) for n in range(NCH) for m in range(NM)):
                        ps = ps6b.tile([128, CW], F32, name="ps6bt",
                                       tag="ps6bt")
                        nc.tensor.matmul(
                            ps[:],
                            w_outT[NDT - 1][:, m * 128:(m + 1) * 128],
                            yg_bf[NDT - 1][:, n * CW:(n + 1) * CW],
                            start=True, stop=True)
                        ot = p6b.tile([128, CW], BF16, name="otb",
                                      tag="otb", bufs=6)
                        if i % 2 == 0:
                            nc.scalar.copy(ot[:], ps[:])
                        else:
                            nc.vector.tensor_copy(ot[:], ps[:])
                        nc.gpsimd.dma_start(
                            dram["out_part5"][m * 128:(m + 1) * 128,
                                              n * CW:(n + 1) * CW], ot[:])


def _stages_123(nc, tc, dram, dbg, wp, env):
    hold = env["hold"]
    dtT_bf = env["dtT_bf"]
    conv_b = env["conv_b"]
    w_xT = env["w_xT"]
    bc_scr = env["bc_scr"]
    z_scr = env["z_scr"]
    xc_scr = env["xc_scr"]
    cc_in_dt = env["cc_in_dt"]
    cc_out_dt = env["cc_out_dt"]
    cc_in_bc = env["cc_in_bc"]
    cc_out_bc = env["cc_out_bc"]
    LPAD = L + 3
    NDT_X = NDT if USE_CC else NDT_FULL

    with tc.tile_pool(name="pre3", bufs=1) as p3, \
         tc.tile_pool(name="ps_a", bufs=2, space="PSUM") as psa:
        xc_bf = [p3.tile([128, L], BF16, name=f"xc{r}", tag=f"xc{r}")
                 for r in range(NDT_X)]
        uT = [p3.tile([128, L], BF16, name=f"uT{k}", tag=f"uT{k}")
              for k in range(NK)]
        for k in range(NK):
            nc.sync.dma_start(uT[k][:],
                              dram["uT"][k * 128:(k + 1) * 128, :])
        w_in_zT = [p3.tile([128, HALF], BF16, name=f"wiz{k}",
                           tag=f"wiz{k}") for k in range(NK)]
        for k in range(NK):
            nc.sync.dma_start(w_in_zT[k][:],
                              dram["w_in_zT"][k * 128:(k + 1) * 128, :])
        with tc.tile_pool(name="pre12", bufs=1) as p12:
            WIX_W = NDT_X * 128
            w_in_xT = [p12.tile([128, WIX_W], BF16, name=f"wix{k}",
                                tag=f"wix{k}") for k in range(NK)]
            for k in range(NK):
                nc.sync.dma_start(
                    w_in_xT[k][:],
                    dram["w_in_xT"][k * 128:(k + 1) * 128, 0:WIX_W])
            cw_cols = env["cw_cols"]

            # ---- stages 1+2 fused per d-tile: in-proj -> conv -> silu.
            # The depthwise conv runs on VectorE (idle during the head)
            # as shifted tensor-scalar ops with per-partition tap
            # weights, freeing the PE phase to reach x_dbl/AllReduce
            # ~20us earlier. Accumulation in fp32, one rounding at the
            # final silu. ----
            for r in range(NDT_X):
                xr = p12.tile([128, LPAD], BF16, name="xr", tag="xr",
                              bufs=2)
                nc.vector.memset(xr[:, 0:3], 0.0)
                for n in range(NCH):
                    ps = psa.tile([128, CW], F32, name="psa", tag="psa")
                    for k in range(NK):
                        nc.tensor.matmul(
                            ps[:], w_in_xT[k][:, r * 128:(r + 1) * 128],
                            uT[k][:, n * CW:(n + 1) * CW],
                            start=(k == 0), stop=(k == NK - 1))
                    nc.vector.tensor_copy(
                        xr[:, 3 + n * CW:3 + (n + 1) * CW], ps[:])
                acc = p12.tile([128, L], F32, name="cacc", tag="cacc",
                               bufs=2)
                nc.vector.tensor_scalar(acc[:], xr[:, 0:L],
                                        cw_cols[r][:, 0:1], None, OP.mult)
                for j in range(1, D_CONV):
                    nc.vector.scalar_tensor_tensor(
                        acc[:], xr[:, j:j + L], cw_cols[r][:, j:j + 1],
                        acc[:], OP.mult, OP.add)
                nc.scalar.activation(xc_bf[r][:], acc[:], AF.Silu,
                                     bias=conv_b[r][:], scale=1.0)
            for r in range(NDT):
                nc.sync.dma_start(xc_scr[r][:], xc_bf[r][:])

        # ---- stage 3: x_dbl (partial if USE_CC, then AllReduce) ----
        xdbl_f = p3.tile([NXP, L], F32, name="xdbl_f", tag="xdbl_f")
        for n in range(NCH):
            ps = psa.tile([NXP, CW], F32, name="ps3", tag="ps3")
            for k in range(NDT_X):
                nc.tensor.matmul(ps[:], w_xT[k][:],
                                 xc_bf[k][:, n * CW:(n + 1) * CW],
                                 start=(k == 0), stop=(k == NDT_X - 1))
            nc.vector.tensor_copy(xdbl_f[:, n * CW:(n + 1) * CW], ps[:])

        if USE_CC:
            # pairwise AllReduce, split so the dt rows (which gate the
            # delta->dA critical chain) land first; B/C rows follow and
            # are only needed slightly later by the b2/c2 broadcasts
            nc.sync.dma_start(cc_in_dt[:], xdbl_f[0:DT_RANK, :])
            nc.sync.dma_start(cc_in_bc[:], xdbl_f[64:NXP, :])
            nc.gpsimd.collective_compute(
                "AllReduce", mybir.AluOpType.add,
                replica_groups=[[0, 1], [2, 3], [4, 5], [6, 7]],
                ins=[cc_in_dt[:]], outs=[cc_out_dt[:]])
            nc.gpsimd.collective_compute(
                "AllReduce", mybir.AluOpType.add,
                replica_groups=[[0, 1], [2, 3], [4, 5], [6, 7]],
                ins=[cc_in_bc[:]], outs=[cc_out_bc[:]])
            xsel_dt = p3.tile([DT_RANK, L], F32, name="xsel_dt",
                              tag="xsel_dt")
            nc.sync.dma_start(xsel_dt[:], cc_out_dt[:])
            xsel_bc = p3.tile([2 * D_STATE, L], F32, name="xsel_bc",
                              tag="xsel_bc")
            nc.sync.dma_start(xsel_bc[:], cc_out_bc[:])
            dt_rows, bc_rows = xsel_dt[:], xsel_bc[:]
        else:
            dt_rows = xdbl_f[0:DT_RANK, :]
            bc_rows = xdbl_f[64:NXP, :]

        nc.scalar.copy(dtT_bf[:], dt_rows)
        bcb = p3.tile([2 * D_STATE, L], BF16, name="bcb", tag="bcb")
        nc.scalar.copy(bcb[:], bc_rows)
        nc.sync.dma_start(bc_scr[:], bcb[:])
        if dbg == 1:
            nc.sync.dma_start(dram["xdbl_dbg"][:], xdbl_use[:])

        # z half -> silu -> spill gz (overlaps scan start on PE/ACT)
        for r in range(NDT):
            zt = p3.tile([128, L], BF16, name="zt", tag="zt", bufs=2)
            for n in range(NCH):
                ps = psa.tile([128, CW], F32, name="psz", tag="psz",
                              bufs=2)
                for k in range(NK):
                    nc.tensor.matmul(
                        ps[:], w_in_zT[k][:, r * 128:(r + 1) * 128],
                        uT[k][:, n * CW:(n + 1) * CW],
                        start=(k == 0), stop=(k == NK - 1))
                nc.vector.tensor_copy(zt[:, n * CW:(n + 1) * CW], ps[:])
            gzt = p3.tile([128, L], BF16, name="gzt", tag="gzt", bufs=2)
            nc.scalar.activation(gzt[:], zt[:], AF.Silu)
            nc.sync.dma_start(z_scr[r][:], gzt[:])

        if dbg == 1:
            for r in range(NDT_X):
                xcd = p3.tile([128, L], F32, name="xcd", tag="xcd", bufs=2)
                nc.vector.tensor_copy(xcd[:], xc_bf[r][:])
                nc.sync.dma_start(dram["xc_dbg"][r * 128:(r + 1) * 128, :],
                                  xcd[:])


def _scan_stage(nc, tc, dram, dbg, wp, env, passA=None):
    passA = list(passA or [])
    dtT_bf = env["dtT_bf"]
    yg_bf = env["yg_bf"]
    bc_scr = env["bc_scr"]
    z_scr = env["z_scr"]
    xc_scr = env["xc_scr"]
    w_dtT = env["w_dtT"]
    A_col = env["A_col"]
    b_dt = env["b_dt"]
    dp_diag = env["dp_diag"]
    idn = env["idn"]

    with tc.tile_pool(name="scanp", bufs=1) as sp, \
         tc.tile_pool(name="ps_mm4", bufs=2, space="PSUM") as ps4, \
         tc.tile_pool(name="ps_y", bufs=1, space="PSUM") as psy:
        for r in range(NDT):
            # ---- delta = softplus(zb), zb = dt @ W_dt.T + b_dt.
            # zb is empirically in [-8, 2] so ln(1+exp(zb)) is safe in
            # fp32; both stages run on ScalarE (fp32 copy feeds the dA
            # exps; bf16 copy feeds du) ----
            delta = sp.tile([128, L], F32, name="delta", tag="delta",
                            bufs=2)
            delta_bf = sp.tile([128, L], BF16, name="delta_bf",
                               tag="delta_bf", bufs=1)
            # batch ACT functions (4x Exp, then Ln, then Copy) so the
            # scalar engine switches activation tables at most twice per
            # r instead of per chunk (each load costs ~1.3us on the
            # delta->dA critical chain)
            tE = sp.tile([128, L], F32, name="tE", tag="tE", bufs=1)
            for n in range(NCH):
                ps = ps4.tile([128, CW], F32, name="ps4t", tag="ps4t")
                nc.tensor.matmul(ps[:], w_dtT[:, r * 128:(r + 1) * 128],
                                 dtT_bf[:, n * CW:(n + 1) * CW],
                                 start=True, stop=True)
                nc.scalar.activation(tE[:, n * CW:(n + 1) * CW], ps[:],
                                     AF.Exp, bias=b_dt[r][:], scale=1.0)
            nc.scalar.activation(delta[:], tE[:], AF.Ln, bias=1.0,
                                 scale=1.0)
            nc.scalar.copy(delta_bf[:], delta[:])
            if dbg == 1:
                nc.sync.dma_start(
                    dram["delta_dbg"][r * 128:(r + 1) * 128, :], delta[:])

            # ---- du = delta * xc ----
            xcr = sp.tile([128, L], BF16, name="xcr", tag="xcr", bufs=2)
            nc.sync.dma_start(xcr[:], xc_scr[r][:])
            du = sp.tile([128, L], BF16, name="du", tag="du", bufs=2)
            nc.vector.tensor_tensor(du[:], delta_bf[:], xcr[:], OP.mult)

            yp = [psy.tile([128, CW], F32, name=f"yp{n}", tag=f"yp{n}")
                  for n in range(NCH)]

            # states processed in fused pairs (s0, s1): one [128, 2L]
            # tile per quantity; scan concatenation is exact because
            # dA2[:, L] (s1's t=0 coefficient, unused by the true
            # recurrence) is pinned to 0, resetting the running state.
            for p_ in range(D_STATE // 2):
                s0, s1 = 2 * p_, 2 * p_ + 1
                fp = s1 < N_S_F32
                tag = "ef" if fp else "eb"
                dA2 = sp.tile([128, 2 * L], F32 if fp else BF16,
                              name=tag, tag=tag, bufs=1 if fp else 2)
                nc.scalar.activation(dA2[:, 0:L], delta[:], AF.Exp,
                                     bias=0.0, scale=A_col[r][:, s0:s0 + 1])
                nc.vector.memset(dA2[:, L:L + 1], 0.0)
                nc.scalar.activation(dA2[:, L + 1:2 * L], delta[:, 1:L],
                                     AF.Exp, bias=0.0,
                                     scale=A_col[r][:, s1:s1 + 1])
                b2 = sp.tile([128, 2 * L], BF16, name="b2", tag="b2",
                             bufs=2)
                nc.sync.dma_start(
                    b2[:, 0:L], bc_scr[s0:s0 + 1, :].broadcast_to((128, L)))
                nc.sync.dma_start(
                    b2[:, L:2 * L],
                    bc_scr[s1:s1 + 1, :].broadcast_to((128, L)))
                c2 = sp.tile([128, 2 * L], BF16, name="c2", tag="c2",
                             bufs=2)
                nc.sync.dma_start(
                    c2[:, 0:L], bc_scr[D_STATE + s0:D_STATE + s0 + 1, :]
                    .broadcast_to((128, L)))
                nc.sync.dma_start(
                    c2[:, L:2 * L],
                    bc_scr[D_STATE + s1:D_STATE + s1 + 1, :]
                    .broadcast_to((128, L)))
                dbu2 = sp.tile([128, 2 * L], BF16, name="dbu2", tag="dbu2",
                               bufs=2)
                nc.vector.tensor_tensor(
                    dbu2[:].rearrange("p (two l) -> p two l", two=2),
                    du[:].unsqueeze(1).broadcast_to((128, 2, L)),
                    b2[:].rearrange("p (two l) -> p two l", two=2),
                    OP.mult)
                h2 = sp.tile([128, 2 * L], BF16, name="h2", tag="h2",
                             bufs=2)
                nc.vector.tensor_tensor_scan(h2[:], dA2[:], dbu2[:], 0.0,
                                             OP.mult, OP.add)
                ws2 = sp.tile([128, 2 * L], BF16, name="ws2", tag="ws2",
                              bufs=2)
                nc.vector.tensor_tensor(ws2[:], h2[:], c2[:], OP.mult)
                for si, s in ((0, s0), (1, s1)):
                    for n in range(NCH):
                        nc.tensor.matmul(
                            yp[n][:], idn[:],
                            ws2[:, si * L + n * CW:si * L + (n + 1) * CW],
                            start=(s == 0), stop=False)
                # drip out-proj pass A into the PE queue while the last
                # r-tile's scan occupies VectorE; front-loaded so the
                # skip/gating matmuls are not queued behind it
                if r == NDT - 1 and p_ < 5:
                    for _ in range(5):
                        if passA:
                            passA.pop(0)()
            # skip term
            for n in range(NCH):
                nc.tensor.matmul(yp[n][:], dp_diag[r][:],
                                 xcr[:, n * CW:(n + 1) * CW],
                                 start=False, stop=True)
            # gate with silu(z) (precomputed gz)
            gz = sp.tile([128, L], BF16, name="gz", tag="gz", bufs=1)
            nc.sync.dma_start(gz[:], z_scr[r][:])
            for n in range(NCH):
                nc.vector.tensor_tensor(yg_bf[r][:, n * CW:(n + 1) * CW],
                                        yp[n][:],
                                        gz[:, n * CW:(n + 1) * CW],
                                        OP.mult)


# ======================= host side =======================

def _prep_core_inputs(inputs, b, rev, h):
    hs = np.asarray(inputs["hidden_states"])
    W_in = np.asarray(inputs["W_in"])
    conv_w = np.asarray(inputs["conv_w"])[:, 0, :]
    conv_b = np.asarray(inputs["conv_b"])
    W_x = np.asarray(inputs["W_x"])
    W_dt = np.asarray(inputs["W_dt"])
    b_dt = np.asarray(inputs["b_dt"])
    A = -np.exp(np.asarray(inputs["A_log"], np.float64)).astype(np.float32)
    Dp = np.asarray(inputs["Dp"])
    W_out = np.asarray(inputs["W_out"])

    lo, hi = h * HALF, (h + 1) * HALF
    perm = np.r_[lo:hi, (0 if h else HALF):(HALF if h else D_INNER)]

    u = hs[b]
    if rev:
        u = u[::-1]
    uT = np.ascontiguousarray(u.T).astype(BF_NP)

    W_in_x = W_in[0:D_INNER][perm]
    W_in_z = W_in[D_INNER + lo:D_INNER + hi]
    conv_wp = conv_w[perm]
    conv_bp = conv_b[perm].reshape(-1, 1).astype(np.float32)
    W_xp = W_x[:, perm]
    W_xpad = np.zeros((NXP, W_xp.shape[1]), W_xp.dtype)
    W_xpad[0:DT_RANK] = W_xp[0:DT_RANK]
    W_xpad[64:96] = W_xp[DT_RANK:NXD]

    idx = np.arange(128)
    dp_diag = np.zeros((NDT * 128, 128), np.float32)
    for r in range(NDT):
        dp_diag[r * 128 + idx, idx] = Dp[lo + r * 128: lo + (r + 1) * 128]

    return {
        "uT": uT,
        "w_in_xT": np.ascontiguousarray(W_in_x.T).astype(BF_NP),
        "w_in_zT": np.ascontiguousarray(W_in_z.T).astype(BF_NP),
        "conv_cols": np.ascontiguousarray(conv_wp).astype(np.float32),
        "conv_b": conv_bp,
        "w_xT": np.ascontiguousarray(W_xpad.T).astype(BF_NP),
        "w_dtT": np.ascontiguousarray(W_dt[lo:hi].T).astype(BF_NP),
        "b_dt": b_dt[lo:hi].reshape(-1, 1).astype(np.float32),
        "A_half": A[lo:hi].astype(np.float32),
        "dp_diag": dp_diag.astype(BF_NP),
        "idn": np.eye(128, dtype=np.float32).astype(BF_NP),
        "w_outT": np.ascontiguousarray(W_out[:, lo:hi].T).astype(BF_NP),
    }


_CACHE = {}


def kernel(**inputs):
    if "prog" not in _CACHE:
        _CACHE["prog"] = build_program(0)
    nc = _CACHE["prog"]

    in_maps = []
    for c in range(8):
        b, rev, h = c >> 2, (c >> 1) & 1, c & 1
        in_maps.append(_prep_core_inputs(inputs, b, rev, h))
    res = run_bass_kernel_spmd(nc, in_maps, list(range(8)))

    out = np.zeros((BATCH, L, D_MODEL), np.float32)
    for c in range(8):
        b, rev, h = c >> 2, (c >> 1) & 1, c & 1
        part = (res.results[c]["out_part"]
                + res.results[c]["out_part5"].astype(np.float32)).T
        if rev:
            part = part[::-1]
        out[b] += part
    return out



# revision 40
# speedup vs baseline: 1.0272x; 1.0272x over previous
"""BiMamba forward kernel for 8 TRN2 NeuronCores.

Sharding: core c = (batch b, direction dir, d_inner half h); the host
pre-flips reverse-direction inputs in time so the device program is
identical (purely causal) on all cores. Each core produces two partial
output projections [d_model, L] (r-tiles 0..4 and r-tile 5); the host
sums them and the four core-partials per batch element (unflipping the
reverse ones). A host-side channel permutation puts this core's d_inner
half in x-path tiles 0..5 so the single SPMD program needs no per-core
branches.

Device layout: channels on partitions, time on the free dim. The scan
is hardware tensor_tensor_scan (h = dA*h + dBu along time). All scan-
phase elementwise work runs on VectorE only (GpSimd shares an SBUF port
with the DVE, so offloading there halves DVE throughput); everything is
bf16 SBUF step-1 to hit the DVE 2x perf mode. States are processed in
fused pairs: one [128, 2L] tile per quantity, with dA's column L pinned
to zero so the concatenated scan resets exactly at the pair boundary
(h_0 never reads dA_0). delta = softplus(dt@W_dt+b_dt) is computed
entirely on ScalarE as Ln(1 + Exp(zb)) -- zb is empirically in [-8, 2]
so the naive form is safe -- with ACT calls batched per function to
minimize activation-table reloads. x_dbl partials are pairwise
AllReduced (replica groups [2p, 2p+1]), split into dt rows and B/C rows
so the dt rows that gate the delta->dA chain land first. The Dp-skip
and sum-over-states accumulate in PSUM via identity/diagonal matmuls on
TensorE; the causal conv runs on TensorE as 4 diagonal matmuls over
shifted views. Out-proj pass A (r 0..4) is drip-fed into the PE queue
while the last r-tile's scan still occupies VectorE; pass B (r=5, bf16)
is the only post-scan work and runs in a deep-buffered
matmul/copy/DMA pipeline.
"""
import numpy as np
import ml_dtypes

import concourse.bass as bass
import concourse.tile as tile
from concourse import bacc, mybir
from concourse.bass_utils import run_bass_kernel_spmd

D_MODEL = 768
D_INNER = 1536
D_STATE = 16
D_CONV = 4
DT_RANK = 48
BATCH = 2
SEQLEN = 2048

HALF = D_INNER // 2
NDT = HALF // 128            # 6 half d-tiles
NDT_FULL = D_INNER // 128    # 12 full d-tiles
NK = D_MODEL // 128          # 6 k-tiles over d_model
L = SEQLEN
NCH = 4
CW = L // NCH                # 512
NXD = DT_RANK + 2 * D_STATE  # 80
NXP = 96                     # x_dbl rows: dt at 0..47, B/C at 64..95
NM = D_MODEL // 128          # 6 out-proj row tiles

F32 = mybir.dt.float32
BF16 = mybir.dt.bfloat16
BF_NP = ml_dtypes.bfloat16

N_S_F32 = 4                                         # fp32 decay planes
USE_CC = True    # pairwise x_dbl via pairwise AllReduce

AF = mybir.ActivationFunctionType
OP = mybir.AluOpType


def build_program(debug_stage=0):
    nc = bacc.Bacc("TRN2", target_bir_lowering=False, debug=False,
                   num_devices=8)
    dram = {}

    def din(name, shape, dt):
        dram[name] = nc.dram_tensor(name, list(shape), dt,
                                    kind="ExternalInput").ap()

    def dout(name, shape, dt):
        dram[name] = nc.dram_tensor(name, list(shape), dt,
                                    kind="ExternalOutput").ap()

    din("uT", (D_MODEL, L), BF16)
    din("w_in_xT", (D_MODEL, D_INNER), BF16)
    din("w_in_zT", (D_MODEL, HALF), BF16)
    din("conv_diag", (NDT_FULL * D_CONV * 128, 128), BF16)
    din("conv_b", (D_INNER, 1), F32)
    din("w_xT", (D_INNER, NXP), BF16)
    din("w_dtT", (DT_RANK, HALF), BF16)
    din("b_dt", (HALF, 1), F32)
    din("A_half", (HALF, D_STATE), F32)
    din("dp_diag", (NDT * 128, 128), BF16)
    din("idn", (128, 128), BF16)
    din("w_outT", (HALF, D_MODEL), BF16)

    if debug_stage == 1:
        dout("xc_dbg", (HALF if USE_CC else D_INNER, L), F32)
        dout("delta_dbg", (HALF, L), F32)
        dout("xdbl_dbg", (NXP, L), F32)
    dout("out_part", (D_MODEL, L), F32)
    dout("out_part5", (D_MODEL, L), BF16)

    with tile.TileContext(nc) as tc:
        _body_once(nc, tc, dram, debug_stage)
    nc.compile()
    return nc


def _body_once(nc, tc, dram, dbg):
    with tc.tile_pool(name="wpool", bufs=1) as wp, \
         tc.tile_pool(name="dramp", bufs=1, space="DRAM") as dp_pool:

        # ---- DRAM scratch (tracked tiles) ----
        bc_scr = dp_pool.tile([2 * D_STATE, L], BF16, name="bc_scr")
        cc_in_dt = dp_pool.tile([DT_RANK, L], F32, name="cc_in_dt")
        cc_out_dt = dp_pool.tile([DT_RANK, L], F32, name="cc_out_dt")
        cc_in_bc = dp_pool.tile([2 * D_STATE, L], F32, name="cc_in_bc")
        cc_out_bc = dp_pool.tile([2 * D_STATE, L], F32, name="cc_out_bc")
        z_scr = [dp_pool.tile([128, L], BF16, name=f"z_scr{r}")
                 for r in range(NDT)]
        xc_scr = [dp_pool.tile([128, L], BF16, name=f"xc_scr{r}")
                  for r in range(NDT)]

        # ---- persistent small weights ----
        idn = wp.tile([128, 128], BF16, name="idn")
        nc.sync.dma_start(idn[:], dram["idn"][:])
        dp_diag = [wp.tile([128, 128], BF16, name=f"dpd{r}")
                   for r in range(NDT)]
        A_col = [wp.tile([128, D_STATE], F32, name=f"acol{r}")
                 for r in range(NDT)]
        b_dt = [wp.tile([128, 1], F32, name=f"bdt{r}") for r in range(NDT)]
        conv_b = [wp.tile([128, 1], F32, name=f"cvb{r}")
                  for r in range(NDT_FULL)]
        for r in range(NDT):
            nc.sync.dma_start(dp_diag[r][:],
                              dram["dp_diag"][r * 128:(r + 1) * 128, :])
            nc.sync.dma_start(A_col[r][:],
                              dram["A_half"][r * 128:(r + 1) * 128, :])
            nc.sync.dma_start(b_dt[r][:],
                              dram["b_dt"][r * 128:(r + 1) * 128, :])
        for r in range(NDT_FULL):
            nc.sync.dma_start(conv_b[r][:],
                              dram["conv_b"][r * 128:(r + 1) * 128, :])
        w_dtT = wp.tile([DT_RANK, HALF], BF16, name="w_dtT")
        nc.sync.dma_start(w_dtT[:], dram["w_dtT"][:])
        w_outT = [wp.tile([128, D_MODEL], BF16, name=f"wout{r}")
                  for r in range(NDT)]
        for r in range(NDT):
            nc.sync.dma_start(w_outT[r][:],
                              dram["w_outT"][r * 128:(r + 1) * 128, :])
        NDT_X = NDT if USE_CC else NDT_FULL
        w_xT = [wp.tile([128, NXP], BF16, name=f"wx{k}")
                for k in range(NDT_X)]
        for k in range(NDT_X):
            nc.sync.dma_start(w_xT[k][:],
                              dram["w_xT"][k * 128:(k + 1) * 128, :])

        with tc.tile_pool(name="hold", bufs=1) as hold:
            dtT_bf = hold.tile([DT_RANK, L], BF16, name="dtT_bf")
            yg_bf = [hold.tile([128, L], BF16, name=f"yg{r}")
                     for r in range(NDT)]

            _stages_123(nc, tc, dram, dbg, wp, locals())

            # ---------- stage 6: out-proj, two passes ----------
            # pass A (r=0..4) is drip-fed into the PE queue while the
            # r=5 scan still runs on VectorE; pass B (r=5 only) is the
            # only post-scan work. The host sums the two partials.
            with tc.tile_pool(name="op6", bufs=1) as p6, \
                 tc.tile_pool(name="ps6", bufs=2, space="PSUM") as ps6:
                def _mk_passA(m, n):
                    def go():
                        ps = ps6.tile([128, CW], F32, name="ps6t",
                                      tag="ps6t")
                        for r in range(NDT - 1):
                            nc.tensor.matmul(
                                ps[:], w_outT[r][:, m * 128:(m + 1) * 128],
                                yg_bf[r][:, n * CW:(n + 1) * CW],
                                start=(r == 0), stop=(r == NDT - 2))
                        ot = p6.tile([128, CW], F32, name="ot", tag="ot",
                                     bufs=2)
                        nc.scalar.copy(ot[:], ps[:])
                        nc.sync.dma_start(
                            dram["out_part"][m * 128:(m + 1) * 128,
                                             n * CW:(n + 1) * CW], ot[:])
                    return go
                passA = [_mk_passA(m, n)
                         for m in range(NM) for n in range(NCH)]
                _scan_stage(nc, tc, dram, dbg, wp, locals(), passA)

                # pass B: deep-buffered pipeline (scan PSUM pools are
                # closed by now), copies alternating scalar/vector
                with tc.tile_pool(name="op6b", bufs=1) as p6b, \
                     tc.tile_pool(name="ps6b", bufs=6,
                                  space="PSUM") as ps6b:
                    for i, (m, n) in enumerate(
                            (m, n) for n in range(NCH) for m in range(NM)):
                        ps = ps6b.tile([128, CW], F32, name="ps6bt",
                                       tag="ps6bt")
                        nc.tensor.matmul(
                            ps[:],
                            w_outT[NDT - 1][:, m * 128:(m + 1) * 128],
                            yg_bf[NDT - 1][:, n * CW:(n + 1) * CW],
                            start=True, stop=True)
                        ot = p6b.tile([128, CW], BF16, name="otb",
                                      tag="otb", bufs=6)
                        if i % 2 == 0:
                            nc.scalar.copy(ot[:], ps[:])
                        else:
                            nc.vector.tensor_copy(ot[:], ps[:])
                        nc.gpsimd.dma_start(
                            dram["out_part5"][m * 128:(m + 1) * 128,
                                              n * CW:(n + 1) * CW], ot[:])


def _stages_123(nc, tc, dram, dbg, wp, env):
    hold = env["hold"]
    dtT_bf = env["dtT_bf"]
    conv_b = env["conv_b"]
    w_xT = env["w_xT"]
    bc_scr = env["bc_scr"]
    z_scr = env["z_scr"]
    xc_scr = env["xc_scr"]
    cc_in_dt = env["cc_in_dt"]
    cc_out_dt = env["cc_out_dt"]
    cc_in_bc = env["cc_in_bc"]
    cc_out_bc = env["cc_out_bc"]
    LPAD = L + 3
    NDT_X = NDT if USE_CC else NDT_FULL

    with tc.tile_pool(name="pre3", bufs=1) as p3, \
         tc.tile_pool(name="ps_a", bufs=2, space="PSUM") as psa:
        xc_bf = [p3.tile([128, L], BF16, name=f"xc{r}", tag=f"xc{r}")
                 for r in range(NDT_X)]
        uT = [p3.tile([128, L], BF16, name=f"uT{k}", tag=f"uT{k}")
              for k in range(NK)]
        for k in range(NK):
            nc.sync.dma_start(uT[k][:],
                              dram["uT"][k * 128:(k + 1) * 128, :])
        w_in_zT = [p3.tile([128, HALF], BF16, name=f"wiz{k}",
                           tag=f"wiz{k}") for k in range(NK)]
        for k in range(NK):
            nc.sync.dma_start(w_in_zT[k][:],
                              dram["w_in_zT"][k * 128:(k + 1) * 128, :])
        with tc.tile_pool(name="pre12", bufs=1) as p12:
            WIX_W = NDT_X * 128
            w_in_xT = [p12.tile([128, WIX_W], BF16, name=f"wix{k}",
                                tag=f"wix{k}") for k in range(NK)]
            for k in range(NK):
                nc.sync.dma_start(
                    w_in_xT[k][:],
                    dram["w_in_xT"][k * 128:(k + 1) * 128, 0:WIX_W])
            conv_diag = [p12.tile([128, 128], BF16, name=f"cvd{i}",
                                  tag=f"cvd{i}")
                         for i in range(NDT_X * D_CONV)]
            for i in range(NDT_X * D_CONV):
                nc.sync.dma_start(conv_diag[i][:],
                                  dram["conv_diag"][i * 128:(i + 1) * 128, :])

            # ---- stages 1+2 fused per d-tile: in-proj -> conv -> silu ----
            for r in range(NDT_X):
                xr = p12.tile([128, LPAD], BF16, name="xr", tag="xr",
                              bufs=2)
                nc.vector.memset(xr[:, 0:3], 0.0)
                for n in range(NCH):
                    ps = psa.tile([128, CW], F32, name="psa", tag="psa")
                    for k in range(NK):
                        nc.tensor.matmul(
                            ps[:], w_in_xT[k][:, r * 128:(r + 1) * 128],
                            uT[k][:, n * CW:(n + 1) * CW],
                            start=(k == 0), stop=(k == NK - 1))
                    nc.vector.tensor_copy(
                        xr[:, 3 + n * CW:3 + (n + 1) * CW], ps[:])
                for n in range(NCH):
                    ps = psa.tile([128, CW], F32, name="psa", tag="psa")
                    for j in range(D_CONV):
                        nc.tensor.matmul(
                            ps[:], conv_diag[r * D_CONV + j][:],
                            xr[:, n * CW + j:n * CW + j + CW],
                            start=(j == 0), stop=(j == D_CONV - 1))
                    nc.scalar.activation(xc_bf[r][:, n * CW:(n + 1) * CW],
                                         ps[:], AF.Silu,
                                         bias=conv_b[r][:], scale=1.0)
            for r in range(NDT):
                nc.sync.dma_start(xc_scr[r][:], xc_bf[r][:])

        # ---- stage 3: x_dbl (partial if USE_CC, then AllReduce) ----
        xdbl_f = p3.tile([NXP, L], F32, name="xdbl_f", tag="xdbl_f")
        for n in range(NCH):
            ps = psa.tile([NXP, CW], F32, name="ps3", tag="ps3")
            for k in range(NDT_X):
                nc.tensor.matmul(ps[:], w_xT[k][:],
                                 xc_bf[k][:, n * CW:(n + 1) * CW],
                                 start=(k == 0), stop=(k == NDT_X - 1))
            nc.vector.tensor_copy(xdbl_f[:, n * CW:(n + 1) * CW], ps[:])

        if USE_CC:
            # pairwise AllReduce, split so the dt rows (which gate the
            # delta->dA critical chain) land first; B/C rows follow and
            # are only needed slightly later by the b2/c2 broadcasts
            nc.sync.dma_start(cc_in_dt[:], xdbl_f[0:DT_RANK, :])
            nc.sync.dma_start(cc_in_bc[:], xdbl_f[64:NXP, :])
            nc.gpsimd.collective_compute(
                "AllReduce", mybir.AluOpType.add,
                replica_groups=[[0, 1], [2, 3], [4, 5], [6, 7]],
                ins=[cc_in_dt[:]], outs=[cc_out_dt[:]])
            nc.gpsimd.collective_compute(
                "AllReduce", mybir.AluOpType.add,
                replica_groups=[[0, 1], [2, 3], [4, 5], [6, 7]],
                ins=[cc_in_bc[:]], outs=[cc_out_bc[:]])
            xsel_dt = p3.tile([DT_RANK, L], F32, name="xsel_dt",
                              tag="xsel_dt")
            nc.sync.dma_start(xsel_dt[:], cc_out_dt[:])
            xsel_bc = p3.tile([2 * D_STATE, L], F32, name="xsel_bc",
                              tag="xsel_bc")
            nc.sync.dma_start(xsel_bc[:], cc_out_bc[:])
            dt_rows, bc_rows = xsel_dt[:], xsel_bc[:]
        else:
            dt_rows = xdbl_f[0:DT_RANK, :]
            bc_rows = xdbl_f[64:NXP, :]

        nc.scalar.copy(dtT_bf[:], dt_rows)
        bcb = p3.tile([2 * D_STATE, L], BF16, name="bcb", tag="bcb")
        nc.scalar.copy(bcb[:], bc_rows)
        nc.sync.dma_start(bc_scr[:], bcb[:])
        if dbg == 1:
            nc.sync.dma_start(dram["xdbl_dbg"][:], xdbl_use[:])

        # z half -> silu -> spill gz (overlaps scan start on PE/ACT)
        for r in range(NDT):
            zt = p3.tile([128, L], BF16, name="zt", tag="zt", bufs=2)
            for n in range(NCH):
                ps = psa.tile([128, CW], F32, name="psz", tag="psz",
                              bufs=2)
                for k in range(NK):
                    nc.tensor.matmul(
                        ps[:], w_in_zT[k][:, r * 128:(r + 1) * 128],
                        uT[k][:, n * CW:(n + 1) * CW],
                        start=(k == 0), stop=(k == NK - 1))
                nc.vector.tensor_copy(zt[:, n * CW:(n + 1) * CW], ps[:])
            gzt = p3.tile([128, L], BF16, name="gzt", tag="gzt", bufs=2)
            nc.scalar.activation(gzt[:], zt[:], AF.Silu)
            nc.sync.dma_start(z_scr[r][:], gzt[:])

        if dbg == 1:
            for r in range(NDT_X):
                xcd = p3.tile([128, L], F32, name="xcd", tag="xcd", bufs=2)
                nc.vector.tensor_copy(xcd[:], xc_bf[r][:])
                nc.sync.dma_start(dram["xc_dbg"][r * 128:(r + 1) * 128, :],
                                  xcd[:])


def _scan_stage(nc, tc, dram, dbg, wp, env, passA=None):
    passA = list(passA or [])
    dtT_bf = env["dtT_bf"]
    yg_bf = env["yg_bf"]
    bc_scr = env["bc_scr"]
    z_scr = env["z_scr"]
    xc_scr = env["xc_scr"]
    w_dtT = env["w_dtT"]
    A_col = env["A_col"]
    b_dt = env["b_dt"]
    dp_diag = env["dp_diag"]
    idn = env["idn"]

    with tc.tile_pool(name="scanp", bufs=1) as sp, \
         tc.tile_pool(name="ps_mm4", bufs=2, space="PSUM") as ps4, \
         tc.tile_pool(name="ps_y", bufs=1, space="PSUM") as psy:
        for r in range(NDT):
            # ---- delta = softplus(zb), zb = dt @ W_dt.T + b_dt.
            # zb is empirically in [-8, 2] so ln(1+exp(zb)) is safe in
            # fp32; both stages run on ScalarE (fp32 copy feeds the dA
            # exps; bf16 copy feeds du) ----
            delta = sp.tile([128, L], F32, name="delta", tag="delta",
                            bufs=2)
            delta_bf = sp.tile([128, L], BF16, name="delta_bf",
                               tag="delta_bf", bufs=1)
            # batch ACT functions (4x Exp, then Ln, then Copy) so the
            # scalar engine switches activation tables at most twice per
            # r instead of per chunk (each load costs ~1.3us on the
            # delta->dA critical chain)
            tE = sp.tile([128, L], F32, name="tE", tag="tE", bufs=1)
            for n in range(NCH):
                ps = ps4.tile([128, CW], F32, name="ps4t", tag="ps4t")
                nc.tensor.matmul(ps[:], w_dtT[:, r * 128:(r + 1) * 128],
                                 dtT_bf[:, n * CW:(n + 1) * CW],
                                 start=True, stop=True)
                nc.scalar.activation(tE[:, n * CW:(n + 1) * CW], ps[:],
                                     AF.Exp, bias=b_dt[r][:], scale=1.0)
            nc.scalar.activation(delta[:], tE[:], AF.Ln, bias=1.0,
                                 scale=1.0)
            nc.scalar.copy(delta_bf[:], delta[:])
            if dbg == 1:
                nc.sync.dma_start(
                    dram["delta_dbg"][r * 128:(r + 1) * 128, :], delta[:])

            # ---- du = delta * xc ----
            xcr = sp.tile([128, L], BF16, name="xcr", tag="xcr", bufs=2)
            nc.sync.dma_start(xcr[:], xc_scr[r][:])
            du = sp.tile([128, L], BF16, name="du", tag="du", bufs=2)
            nc.vector.tensor_tensor(du[:], delta_bf[:], xcr[:], OP.mult)

            yp = [psy.tile([128, CW], F32, name=f"yp{n}", tag=f"yp{n}")
                  for n in range(NCH)]

            # states processed in fused pairs (s0, s1): one [128, 2L]
            # tile per quantity; scan concatenation is exact because
            # dA2[:, L] (s1's t=0 coefficient, unused by the true
            # recurrence) is pinned to 0, resetting the running state.
            for p_ in range(D_STATE // 2):
                s0, s1 = 2 * p_, 2 * p_ + 1
                fp = s1 < N_S_F32
                tag = "ef" if fp else "eb"
                dA2 = sp.tile([128, 2 * L], F32 if fp else BF16,
                              name=tag, tag=tag, bufs=1 if fp else 2)
                nc.scalar.activation(dA2[:, 0:L], delta[:], AF.Exp,
                                     bias=0.0, scale=A_col[r][:, s0:s0 + 1])
                nc.vector.memset(dA2[:, L:L + 1], 0.0)
                nc.scalar.activation(dA2[:, L + 1:2 * L], delta[:, 1:L],
                                     AF.Exp, bias=0.0,
                                     scale=A_col[r][:, s1:s1 + 1])
                b2 = sp.tile([128, 2 * L], BF16, name="b2", tag="b2",
                             bufs=2)
                nc.sync.dma_start(
                    b2[:, 0:L], bc_scr[s0:s0 + 1, :].broadcast_to((128, L)))
                nc.sync.dma_start(
                    b2[:, L:2 * L],
                    bc_scr[s1:s1 + 1, :].broadcast_to((128, L)))
                c2 = sp.tile([128, 2 * L], BF16, name="c2", tag="c2",
                             bufs=2)
                nc.sync.dma_start(
                    c2[:, 0:L], bc_scr[D_STATE + s0:D_STATE + s0 + 1, :]
                    .broadcast_to((128, L)))
                nc.sync.dma_start(
                    c2[:, L:2 * L],
                    bc_scr[D_STATE + s1:D_STATE + s1 + 1, :]
                    .broadcast_to((128, L)))
                dbu2 = sp.tile([128, 2 * L], BF16, name="dbu2", tag="dbu2",
                               bufs=2)
                nc.vector.tensor_tensor(
                    dbu2[:].rearrange("p (two l) -> p two l", two=2),
                    du[:].unsqueeze(1).broadcast_to((128, 2, L)),
                    b2[:].rearrange("p (two l) -> p two l", two=2),
                    OP.mult)
                h2 = sp.tile([128, 2 * L], BF16, name="h2", tag="h2",
                             bufs=2)
                nc.vector.tensor_tensor_scan(h2[:], dA2[:], dbu2[:], 0.0,
                                             OP.mult, OP.add)
                ws2 = sp.tile([128, 2 * L], BF16, name="ws2", tag="ws2",
                              bufs=2)
                nc.vector.tensor_tensor(ws2[:], h2[:], c2[:], OP.mult)
                for si, s in ((0, s0), (1, s1)):
                    for n in range(NCH):
                        nc.tensor.matmul(
                            yp[n][:], idn[:],
                            ws2[:, si * L + n * CW:si * L + (n + 1) * CW],
                            start=(s == 0), stop=False)
                # drip out-proj pass A into the PE queue while the last
                # r-tile's scan occupies VectorE; front-loaded so the
                # skip/gating matmuls are not queued behind it
                if r == NDT - 1 and p_ < 5:
                    for _ in range(5):
                        if passA:
                            passA.pop(0)()
            # skip term
            for n in range(NCH):
                nc.tensor.matmul(yp[n][:], dp_diag[r][:],
                                 xcr[:, n * CW:(n + 1) * CW],
                                 start=False, stop=True)
            # gate with silu(z) (precomputed gz)
            gz = sp.tile([128, L], BF16, name="gz", tag="gz", bufs=1)
            nc.sync.dma_start(gz[:], z_scr[r][:])
            for n in range(NCH):
                nc.vector.tensor_tensor(yg_bf[r][:, n * CW:(n + 1) * CW],
                                        yp[n][:],
                                        gz[:, n * CW:(n + 1) * CW],
                                        OP.mult)


# ======================= host side =======================

def _prep_core_inputs(inputs, b, rev, h):
    hs = np.asarray(inputs["hidden_states"])
    W_in = np.asarray(inputs["W_in"])
    conv_w = np.asarray(inputs["conv_w"])[:, 0, :]
    conv_b = np.asarray(inputs["conv_b"])
    W_x = np.asarray(inputs["W_x"])
    W_dt = np.asarray(inputs["W_dt"])
    b_dt = np.asarray(inputs["b_dt"])
    A = -np.exp(np.asarray(inputs["A_log"], np.float64)).astype(np.float32)
    Dp = np.asarray(inputs["Dp"])
    W_out = np.asarray(inputs["W_out"])

    lo, hi = h * HALF, (h + 1) * HALF
    perm = np.r_[lo:hi, (0 if h else HALF):(HALF if h else D_INNER)]

    u = hs[b]
    if rev:
        u = u[::-1]
    uT = np.ascontiguousarray(u.T).astype(BF_NP)

    W_in_x = W_in[0:D_INNER][perm]
    W_in_z = W_in[D_INNER + lo:D_INNER + hi]
    conv_wp = conv_w[perm]
    conv_bp = conv_b[perm].reshape(-1, 1).astype(np.float32)
    W_xp = W_x[:, perm]
    W_xpad = np.zeros((NXP, W_xp.shape[1]), W_xp.dtype)
    W_xpad[0:DT_RANK] = W_xp[0:DT_RANK]
    W_xpad[64:96] = W_xp[DT_RANK:NXD]

    conv_diag = np.zeros((NDT_FULL * D_CONV * 128, 128), np.float32)
    idx = np.arange(128)
    for r in range(NDT_FULL):
        for j in range(D_CONV):
            base = (r * D_CONV + j) * 128
            conv_diag[base + idx, idx] = conv_wp[r * 128:(r + 1) * 128, j]

    dp_diag = np.zeros((NDT * 128, 128), np.float32)
    for r in range(NDT):
        dp_diag[r * 128 + idx, idx] = Dp[lo + r * 128: lo + (r + 1) * 128]

    return {
        "uT": uT,
        "w_in_xT": np.ascontiguousarray(W_in_x.T).astype(BF_NP),
        "w_in_zT": np.ascontiguousarray(W_in_z.T).astype(BF_NP),
        "conv_diag": conv_diag.astype(BF_NP),
        "conv_b": conv_bp,
        "w_xT": np.ascontiguousarray(W_xpad.T).astype(BF_NP),
        "w_dtT": np.ascontiguousarray(W_dt[lo:hi].T).astype(BF_NP),
        "b_dt": b_dt[lo:hi].reshape(-1, 1).astype(np.float32),
        "A_half": A[lo:hi].astype(np.float32),
        "dp_diag": dp_diag.astype(BF_NP),
        "idn": np.eye(128, dtype=np.float32).astype(BF_NP),
        "w_outT": np.ascontiguousarray(W_out[:, lo:hi].T).astype(BF_NP),
    }


_CACHE = {}


def kernel(**inputs):
    if "prog" not in _CACHE:
        _CACHE["prog"] = build_program(0)
    nc = _CACHE["prog"]

    in_maps = []
    for c in range(8):
        b, rev, h = c >> 2, (c >> 1) & 1, c & 1
        in_maps.append(_prep_core_inputs(inputs, b, rev, h))
    res = run_bass_kernel_spmd(nc, in_maps, list(range(8)))

    out = np.zeros((BATCH, L, D_MODEL), np.float32)
    for c in range(8):
        b, rev, h = c >> 2, (c >> 1) & 1, c & 1
        part = (res.results[c]["out_part"]
                + res.results[c]["out_part5"].astype(np.float32)).T
        if rev:
            part = part[::-1]
        out[b] += part
    return out



# revision 41
# speedup vs baseline: 1.1960x; 1.1643x over previous
"""BiMamba forward kernel for 8 TRN2 NeuronCores.

Sharding: core c = (batch b, direction dir, d_inner half h); the host
pre-flips reverse-direction inputs in time so the device program is
identical (purely causal) on all cores. Each core produces two partial
output projections [d_model, L] (r-tiles 0..4 and r-tile 5); the host
sums them and the four core-partials per batch element (unflipping the
reverse ones). A host-side channel permutation puts this core's d_inner
half in x-path tiles 0..5 so the single SPMD program needs no per-core
branches.

Device layout: channels on partitions, time on the free dim. The scan
is hardware tensor_tensor_scan (h = dA*h + dBu along time). All scan-
phase elementwise work runs on VectorE only (GpSimd shares an SBUF port
with the DVE, so offloading there halves DVE throughput); everything is
bf16 SBUF step-1 to hit the DVE 2x perf mode. States are processed in
fused pairs: one [128, 2L] tile per quantity, with dA's column L pinned
to zero so the concatenated scan resets exactly at the pair boundary
(h_0 never reads dA_0). delta = softplus(dt@W_dt+b_dt) is computed
entirely on ScalarE as Ln(1 + Exp(zb)) -- zb is empirically in [-8, 2]
so the naive form is safe -- with ACT calls batched per function to
minimize activation-table reloads. x_dbl partials are pairwise
AllReduced (replica groups [2p, 2p+1]), split into dt rows and B/C rows
so the dt rows that gate the delta->dA chain land first. The Dp-skip
and sum-over-states accumulate in PSUM via identity/diagonal matmuls on
TensorE; the causal conv runs on TensorE as 4 diagonal matmuls over
shifted views. Out-proj pass A (r 0..4) is drip-fed into the PE queue
while the last r-tile's scan still occupies VectorE; pass B (r=5, bf16)
is the only post-scan work and runs in a deep-buffered
matmul/copy/DMA pipeline.
"""
import numpy as np
import ml_dtypes

import concourse.bass as bass
import concourse.tile as tile
from concourse import bacc, mybir
from concourse.bass_utils import run_bass_kernel_spmd

D_MODEL = 768
D_INNER = 1536
D_STATE = 16
D_CONV = 4
DT_RANK = 48
BATCH = 2
SEQLEN = 2048

HALF = D_INNER // 2
NDT = HALF // 128            # 6 half d-tiles
NDT_FULL = D_INNER // 128    # 12 full d-tiles
NK = D_MODEL // 128          # 6 k-tiles over d_model
L = SEQLEN
NCH = 4
CW = L // NCH                # 512
NXD = DT_RANK + 2 * D_STATE  # 80
NXP = 96                     # x_dbl rows: dt at 0..47, B/C at 64..95
NM = D_MODEL // 128          # 6 out-proj row tiles

F32 = mybir.dt.float32
BF16 = mybir.dt.bfloat16
BF_NP = ml_dtypes.bfloat16

N_S_F32 = 4                                         # fp32 decay planes
USE_CC = True    # pairwise x_dbl via pairwise AllReduce

AF = mybir.ActivationFunctionType
OP = mybir.AluOpType


def build_program(debug_stage=0):
    nc = bacc.Bacc("TRN2", target_bir_lowering=False, debug=False,
                   num_devices=8)
    dram = {}

    def din(name, shape, dt):
        dram[name] = nc.dram_tensor(name, list(shape), dt,
                                    kind="ExternalInput").ap()

    def dout(name, shape, dt):
        dram[name] = nc.dram_tensor(name, list(shape), dt,
                                    kind="ExternalOutput").ap()

    din("uT", (D_MODEL, L), BF16)
    din("w_in_xT", (D_MODEL, D_INNER), BF16)
    din("w_in_zT", (D_MODEL, HALF), BF16)
    din("conv_diag", (NDT_FULL * D_CONV * 128, 128), BF16)
    din("conv_b", (D_INNER, 1), F32)
    din("w_xT", (D_INNER, NXP), BF16)
    din("w_dtT", (DT_RANK, HALF), BF16)
    din("b_dt", (HALF, 1), F32)
    din("A_half", (HALF, D_STATE), F32)
    din("dp_diag", (NDT * 128, 128), BF16)
    din("idn", (128, 128), BF16)
    din("w_outT", (HALF, D_MODEL), BF16)

    if debug_stage == 1:
        dout("xc_dbg", (HALF if USE_CC else D_INNER, L), F32)
        dout("delta_dbg", (HALF, L), F32)
        dout("xdbl_dbg", (NXP, L), F32)
    dout("out_part", (D_MODEL, L), F32)
    dout("out_part5", (D_MODEL, L), BF16)

    with tile.TileContext(nc) as tc:
        _body_once(nc, tc, dram, debug_stage)
    nc.compile()
    return nc


def _body_once(nc, tc, dram, dbg):
    with tc.tile_pool(name="wpool", bufs=1) as wp, \
         tc.tile_pool(name="dramp", bufs=1, space="DRAM") as dp_pool:

        # ---- DRAM scratch (tracked tiles) ----
        bc_scr = dp_pool.tile([2 * D_STATE, L], BF16, name="bc_scr")
        cc_in_dt = dp_pool.tile([DT_RANK, L], F32, name="cc_in_dt")
        cc_out_dt = dp_pool.tile([DT_RANK, L], F32, name="cc_out_dt")
        cc_in_bc = dp_pool.tile([2 * D_STATE, L], F32, name="cc_in_bc")
        cc_out_bc = dp_pool.tile([2 * D_STATE, L], F32, name="cc_out_bc")
        z_scr = [dp_pool.tile([128, L], BF16, name=f"z_scr{r}")
                 for r in range(NDT)]
        xc_scr = [dp_pool.tile([128, L], BF16, name=f"xc_scr{r}")
                  for r in range(NDT)]

        # ---- persistent small weights ----
        idn = wp.tile([128, 128], BF16, name="idn")
        nc.sync.dma_start(idn[:], dram["idn"][:])
        dp_diag = [wp.tile([128, 128], BF16, name=f"dpd{r}")
                   for r in range(NDT)]
        A_col = [wp.tile([128, D_STATE], F32, name=f"acol{r}")
                 for r in range(NDT)]
        b_dt = [wp.tile([128, 1], F32, name=f"bdt{r}") for r in range(NDT)]
        conv_b = [wp.tile([128, 1], F32, name=f"cvb{r}")
                  for r in range(NDT_FULL)]
        for r in range(NDT):
            nc.sync.dma_start(dp_diag[r][:],
                              dram["dp_diag"][r * 128:(r + 1) * 128, :])
            nc.sync.dma_start(A_col[r][:],
                              dram["A_half"][r * 128:(r + 1) * 128, :])
            nc.sync.dma_start(b_dt[r][:],
                              dram["b_dt"][r * 128:(r + 1) * 128, :])
        for r in range(NDT_FULL):
            nc.sync.dma_start(conv_b[r][:],
                              dram["conv_b"][r * 128:(r + 1) * 128, :])
        w_dtT = wp.tile([DT_RANK, HALF], BF16, name="w_dtT")
        nc.sync.dma_start(w_dtT[:], dram["w_dtT"][:])
        w_outT = [wp.tile([128, D_MODEL], BF16, name=f"wout{r}")
                  for r in range(NDT)]
        for r in range(NDT):
            nc.sync.dma_start(w_outT[r][:],
                              dram["w_outT"][r * 128:(r + 1) * 128, :])
        NDT_X = NDT if USE_CC else NDT_FULL
        w_xT = [wp.tile([128, NXP], BF16, name=f"wx{k}")
                for k in range(NDT_X)]
        for k in range(NDT_X):
            nc.sync.dma_start(w_xT[k][:],
                              dram["w_xT"][k * 128:(k + 1) * 128, :])

        with tc.tile_pool(name="hold", bufs=1) as hold:
            dtT_bf = hold.tile([DT_RANK, L], BF16, name="dtT_bf")
            yg_bf = [hold.tile([128, L], BF16, name=f"yg{r}")
                     for r in range(NDT)]

            _stages_123(nc, tc, dram, dbg, wp, locals())

            # ---------- stage 6: out-proj, two passes ----------
            # pass A (r=0..4) is drip-fed into the PE queue while the
            # r=5 scan still runs on VectorE; pass B (r=5 only) is the
            # only post-scan work. The host sums the two partials.
            with tc.tile_pool(name="op6", bufs=1) as p6, \
                 tc.tile_pool(name="ps6", bufs=2, space="PSUM") as ps6:
                def _mk_passA(m, n):
                    def go():
                        ps = ps6.tile([128, CW], F32, name="ps6t",
                                      tag="ps6t")
                        for r in range(NDT - 1):
                            nc.tensor.matmul(
                                ps[:], w_outT[r][:, m * 128:(m + 1) * 128],
                                yg_bf[r][:, n * CW:(n + 1) * CW],
                                start=(r == 0), stop=(r == NDT - 2))
                        ot = p6.tile([128, CW], F32, name="ot", tag="ot",
                                     bufs=2)
                        nc.scalar.copy(ot[:], ps[:])
                        nc.sync.dma_start(
                            dram["out_part"][m * 128:(m + 1) * 128,
                                             n * CW:(n + 1) * CW], ot[:])
                    return go
                passA = [_mk_passA(m, n)
                         for m in range(NM) for n in range(NCH)]
                _scan_stage(nc, tc, dram, dbg, wp, locals(), passA)

                # pass B: deep-buffered pipeline (scan PSUM pools are
                # closed by now), copies alternating scalar/vector
                with tc.tile_pool(name="op6b", bufs=1) as p6b, \
                     tc.tile_pool(name="ps6b", bufs=6,
                                  space="PSUM") as ps6b:
                    for i, (m, n) in enumerate(
                            (m, n) for n in range(NCH) for m in range(NM)):
                        ps = ps6b.tile([128, CW], F32, name="ps6bt",
                                       tag="ps6bt")
                        nc.tensor.matmul(
                            ps[:],
                            w_outT[NDT - 1][:, m * 128:(m + 1) * 128],
                            yg_bf[NDT - 1][:, n * CW:(n + 1) * CW],
                            start=True, stop=True)
                        ot = p6b.tile([128, CW], BF16, name="otb",
                                      tag="otb", bufs=6)
                        if i % 2 == 0:
                            nc.scalar.copy(ot[:], ps[:])
                        else:
                            nc.vector.tensor_copy(ot[:], ps[:])
                        nc.sync.dma_start(
                            dram["out_part5"][m * 128:(m + 1) * 128,
                                              n * CW:(n + 1) * CW], ot[:])


def _stages_123(nc, tc, dram, dbg, wp, env):
    hold = env["hold"]
    dtT_bf = env["dtT_bf"]
    conv_b = env["conv_b"]
    w_xT = env["w_xT"]
    bc_scr = env["bc_scr"]
    z_scr = env["z_scr"]
    xc_scr = env["xc_scr"]
    cc_in_dt = env["cc_in_dt"]
    cc_out_dt = env["cc_out_dt"]
    cc_in_bc = env["cc_in_bc"]
    cc_out_bc = env["cc_out_bc"]
    LPAD = L + 3
    NDT_X = NDT if USE_CC else NDT_FULL

    with tc.tile_pool(name="pre3", bufs=1) as p3, \
         tc.tile_pool(name="ps_a", bufs=2, space="PSUM") as psa:
        xc_bf = [p3.tile([128, L], BF16, name=f"xc{r}", tag=f"xc{r}")
                 for r in range(NDT_X)]
        uT = [p3.tile([128, L], BF16, name=f"uT{k}", tag=f"uT{k}")
              for k in range(NK)]
        for k in range(NK):
            nc.sync.dma_start(uT[k][:],
                              dram["uT"][k * 128:(k + 1) * 128, :])
        w_in_zT = [p3.tile([128, HALF], BF16, name=f"wiz{k}",
                           tag=f"wiz{k}") for k in range(NK)]
        for k in range(NK):
            nc.sync.dma_start(w_in_zT[k][:],
                              dram["w_in_zT"][k * 128:(k + 1) * 128, :])
        with tc.tile_pool(name="pre12", bufs=1) as p12:
            WIX_W = NDT_X * 128
            w_in_xT = [p12.tile([128, WIX_W], BF16, name=f"wix{k}",
                                tag=f"wix{k}") for k in range(NK)]
            for k in range(NK):
                nc.sync.dma_start(
                    w_in_xT[k][:],
                    dram["w_in_xT"][k * 128:(k + 1) * 128, 0:WIX_W])
            conv_diag = [p12.tile([128, 128], BF16, name=f"cvd{i}",
                                  tag=f"cvd{i}")
                         for i in range(NDT_X * D_CONV)]
            for i in range(NDT_X * D_CONV):
                nc.sync.dma_start(conv_diag[i][:],
                                  dram["conv_diag"][i * 128:(i + 1) * 128, :])

            # ---- stages 1+2 fused per d-tile: in-proj -> conv -> silu ----
            for r in range(NDT_X):
                xr = p12.tile([128, LPAD], BF16, name="xr", tag="xr",
                              bufs=2)
                nc.vector.memset(xr[:, 0:3], 0.0)
                for n in range(NCH):
                    ps = psa.tile([128, CW], F32, name="psa", tag="psa")
                    for k in range(NK):
                        nc.tensor.matmul(
                            ps[:], w_in_xT[k][:, r * 128:(r + 1) * 128],
                            uT[k][:, n * CW:(n + 1) * CW],
                            start=(k == 0), stop=(k == NK - 1))
                    nc.vector.tensor_copy(
                        xr[:, 3 + n * CW:3 + (n + 1) * CW], ps[:])
                for n in range(NCH):
                    ps = psa.tile([128, CW], F32, name="psa", tag="psa")
                    for j in range(D_CONV):
                        nc.tensor.matmul(
                            ps[:], conv_diag[r * D_CONV + j][:],
                            xr[:, n * CW + j:n * CW + j + CW],
                            start=(j == 0), stop=(j == D_CONV - 1))
                    nc.scalar.activation(xc_bf[r][:, n * CW:(n + 1) * CW],
                                         ps[:], AF.Silu,
                                         bias=conv_b[r][:], scale=1.0)
            for r in range(NDT):
                nc.sync.dma_start(xc_scr[r][:], xc_bf[r][:])

        # ---- stage 3: x_dbl (partial if USE_CC, then AllReduce) ----
        xdbl_f = p3.tile([NXP, L], F32, name="xdbl_f", tag="xdbl_f")
        for n in range(NCH):
            ps = psa.tile([NXP, CW], F32, name="ps3", tag="ps3")
            for k in range(NDT_X):
                nc.tensor.matmul(ps[:], w_xT[k][:],
                                 xc_bf[k][:, n * CW:(n + 1) * CW],
                                 start=(k == 0), stop=(k == NDT_X - 1))
            nc.vector.tensor_copy(xdbl_f[:, n * CW:(n + 1) * CW], ps[:])

        if USE_CC:
            # pairwise AllReduce, split so the dt rows (which gate the
            # delta->dA critical chain) land first; B/C rows follow and
            # are only needed slightly later by the b2/c2 broadcasts
            nc.sync.dma_start(cc_in_dt[:], xdbl_f[0:DT_RANK, :])
            nc.sync.dma_start(cc_in_bc[:], xdbl_f[64:NXP, :])
            nc.gpsimd.collective_compute(
                "AllReduce", mybir.AluOpType.add,
                replica_groups=[[0, 1], [2, 3], [4, 5], [6, 7]],
                ins=[cc_in_dt[:]], outs=[cc_out_dt[:]])
            nc.gpsimd.collective_compute(
                "AllReduce", mybir.AluOpType.add,
                replica_groups=[[0, 1], [2, 3], [4, 5], [6, 7]],
                ins=[cc_in_bc[:]], outs=[cc_out_bc[:]])
            xsel_dt = p3.tile([DT_RANK, L], F32, name="xsel_dt",
                              tag="xsel_dt")
            nc.sync.dma_start(xsel_dt[:], cc_out_dt[:])
            xsel_bc = p3.tile([2 * D_STATE, L], F32, name="xsel_bc",
                              tag="xsel_bc")
            nc.sync.dma_start(xsel_bc[:], cc_out_bc[:])
            dt_rows, bc_rows = xsel_dt[:], xsel_bc[:]
        else:
            dt_rows = xdbl_f[0:DT_RANK, :]
            bc_rows = xdbl_f[64:NXP, :]

        nc.scalar.copy(dtT_bf[:], dt_rows)
        bcb = p3.tile([2 * D_STATE, L], BF16, name="bcb", tag="bcb")
        nc.scalar.copy(bcb[:], bc_rows)
        nc.sync.dma_start(bc_scr[:], bcb[:])
        if dbg == 1:
            nc.sync.dma_start(dram["xdbl_dbg"][:], xdbl_use[:])

        # z half -> silu -> spill gz (overlaps scan start on PE/ACT)
        for r in range(NDT):
            zt = p3.tile([128, L], BF16, name="zt", tag="zt", bufs=2)
            for n in range(NCH):
                ps = psa.tile([128, CW], F32, name="psz", tag="psz",
                              bufs=2)
                for k in range(NK):
                    nc.tensor.matmul(
                        ps[:], w_in_zT[k][:, r * 128:(r + 1) * 128],
                        uT[k][:, n * CW:(n + 1) * CW],
                        start=(k == 0), stop=(k == NK - 1))
                nc.vector.tensor_copy(zt[:, n * CW:(n + 1) * CW], ps[:])
            gzt = p3.tile([128, L], BF16, name="gzt", tag="gzt", bufs=2)
            nc.scalar.activation(gzt[:], zt[:], AF.Silu)
            nc.sync.dma_start(z_scr[r][:], gzt[:])

        if dbg == 1:
            for r in range(NDT_X):
                xcd = p3.tile([128, L], F32, name="xcd", tag="xcd", bufs=2)
                nc.vector.tensor_copy(xcd[:], xc_bf[r][:])
                nc.sync.dma_start(dram["xc_dbg"][r * 128:(r + 1) * 128, :],
                                  xcd[:])


def _scan_stage(nc, tc, dram, dbg, wp, env, passA=None):
    passA = list(passA or [])
    dtT_bf = env["dtT_bf"]
    yg_bf = env["yg_bf"]
    bc_scr = env["bc_scr"]
    z_scr = env["z_scr"]
    xc_scr = env["xc_scr"]
    w_dtT = env["w_dtT"]
    A_col = env["A_col"]
    b_dt = env["b_dt"]
    dp_diag = env["dp_diag"]
    idn = env["idn"]

    with tc.tile_pool(name="scanp", bufs=1) as sp, \
         tc.tile_pool(name="ps_mm4", bufs=2, space="PSUM") as ps4, \
         tc.tile_pool(name="ps_y", bufs=1, space="PSUM") as psy:
        for r in range(NDT):
            # ---- delta = softplus(zb), zb = dt @ W_dt.T + b_dt.
            # zb is empirically in [-8, 2] so ln(1+exp(zb)) is safe in
            # fp32; both stages run on ScalarE (fp32 copy feeds the dA
            # exps; bf16 copy feeds du) ----
            delta = sp.tile([128, L], F32, name="delta", tag="delta",
                            bufs=2)
            delta_bf = sp.tile([128, L], BF16, name="delta_bf",
                               tag="delta_bf", bufs=1)
            # batch ACT functions (4x Exp, then Ln, then Copy) so the
            # scalar engine switches activation tables at most twice per
            # r instead of per chunk (each load costs ~1.3us on the
            # delta->dA critical chain)
            tE = sp.tile([128, L], F32, name="tE", tag="tE", bufs=1)
            for n in range(NCH):
                ps = ps4.tile([128, CW], F32, name="ps4t", tag="ps4t")
                nc.tensor.matmul(ps[:], w_dtT[:, r * 128:(r + 1) * 128],
                                 dtT_bf[:, n * CW:(n + 1) * CW],
                                 start=True, stop=True)
                nc.scalar.activation(tE[:, n * CW:(n + 1) * CW], ps[:],
                                     AF.Exp, bias=b_dt[r][:], scale=1.0)
            nc.scalar.activation(delta[:], tE[:], AF.Ln, bias=1.0,
                                 scale=1.0)
            nc.scalar.copy(delta_bf[:], delta[:])
            if dbg == 1:
                nc.sync.dma_start(
                    dram["delta_dbg"][r * 128:(r + 1) * 128, :], delta[:])

            # ---- du = delta * xc ----
            xcr = sp.tile([128, L], BF16, name="xcr", tag="xcr", bufs=2)
            nc.sync.dma_start(xcr[:], xc_scr[r][:])
            du = sp.tile([128, L], BF16, name="du", tag="du", bufs=2)
            nc.vector.tensor_tensor(du[:], delta_bf[:], xcr[:], OP.mult)

            yp = [psy.tile([128, CW], F32, name=f"yp{n}", tag=f"yp{n}")
                  for n in range(NCH)]

            # states processed in fused pairs (s0, s1): one [128, 2L]
            # tile per quantity; scan concatenation is exact because
            # dA2[:, L] (s1's t=0 coefficient, unused by the true
            # recurrence) is pinned to 0, resetting the running state.
            for p_ in range(D_STATE // 2):
                s0, s1 = 2 * p_, 2 * p_ + 1
                fp = s1 < N_S_F32
                tag = "ef" if fp else "eb"
                dA2 = sp.tile([128, 2 * L], F32 if fp else BF16,
                              name=tag, tag=tag, bufs=1 if fp else 2)
                nc.scalar.activation(dA2[:, 0:L], delta[:], AF.Exp,
                                     bias=0.0, scale=A_col[r][:, s0:s0 + 1])
                nc.vector.memset(dA2[:, L:L + 1], 0.0)
                nc.scalar.activation(dA2[:, L + 1:2 * L], delta[:, 1:L],
                                     AF.Exp, bias=0.0,
                                     scale=A_col[r][:, s1:s1 + 1])
                b2 = sp.tile([128, 2 * L], BF16, name="b2", tag="b2",
                             bufs=2)
                nc.sync.dma_start(
                    b2[:, 0:L], bc_scr[s0:s0 + 1, :].broadcast_to((128, L)))
                nc.sync.dma_start(
                    b2[:, L:2 * L],
                    bc_scr[s1:s1 + 1, :].broadcast_to((128, L)))
                c2 = sp.tile([128, 2 * L], BF16, name="c2", tag="c2",
                             bufs=2)
                nc.sync.dma_start(
                    c2[:, 0:L], bc_scr[D_STATE + s0:D_STATE + s0 + 1, :]
                    .broadcast_to((128, L)))
                nc.sync.dma_start(
                    c2[:, L:2 * L],
                    bc_scr[D_STATE + s1:D_STATE + s1 + 1, :]
                    .broadcast_to((128, L)))
                dbu2 = sp.tile([128, 2 * L], BF16, name="dbu2", tag="dbu2",
                               bufs=2)
                nc.vector.tensor_tensor(
                    dbu2[:].rearrange("p (two l) -> p two l", two=2),
                    du[:].unsqueeze(1).broadcast_to((128, 2, L)),
                    b2[:].rearrange("p (two l) -> p two l", two=2),
                    OP.mult)
                h2 = sp.tile([128, 2 * L], BF16, name="h2", tag="h2",
                             bufs=2)
                nc.vector.tensor_tensor_scan(h2[:], dA2[:], dbu2[:], 0.0,
                                             OP.mult, OP.add)
                ws2 = sp.tile([128, 2 * L], BF16, name="ws2", tag="ws2",
                              bufs=2)
                nc.vector.tensor_tensor(ws2[:], h2[:], c2[:], OP.mult)
                for si, s in ((0, s0), (1, s1)):
                    for n in range(NCH):
                        nc.tensor.matmul(
                            yp[n][:], idn[:],
                            ws2[:, si * L + n * CW:si * L + (n + 1) * CW],
                            start=(s == 0), stop=False)
                # drip out-proj pass A into the PE queue while the last
                # r-tile's scan occupies VectorE; front-loaded so the
                # skip/gating matmuls are not queued behind it
                if r == NDT - 1 and p_ < 5:
                    for _ in range(5):
                        if passA:
                            passA.pop(0)()
            # skip term
            for n in range(NCH):
                nc.tensor.matmul(yp[n][:], dp_diag[r][:],
                                 xcr[:, n * CW:(n + 1) * CW],
                                 start=False, stop=True)
            # gate with silu(z) (precomputed gz)
            gz = sp.tile([128, L], BF16, name="gz", tag="gz", bufs=1)
            nc.sync.dma_start(gz[:], z_scr[r][:])
            for n in range(NCH):
                nc.vector.tensor_tensor(yg_bf[r][:, n * CW:(n + 1) * CW],
                                        yp[n][:],
                                        gz[:, n * CW:(n + 1) * CW],
                                        OP.mult)


# ======================= host side =======================

def _prep_core_inputs(inputs, b, rev, h):
    hs = np.asarray(inputs["hidden_states"])
    W_in = np.asarray(inputs["W_in"])
    conv_w = np.asarray(inputs["conv_w"])[:, 0, :]
    conv_b = np.asarray(inputs["conv_b"])
    W_x = np.asarray(inputs["W_x"])
    W_dt = np.asarray(inputs["W_dt"])
    b_dt = np.asarray(inputs["b_dt"])
    A = -np.exp(np.asarray(inputs["A_log"], np.float64)).astype(np.float32)
    Dp = np.asarray(inputs["Dp"])
    W_out = np.asarray(inputs["W_out"])

    lo, hi = h * HALF, (h + 1) * HALF
    perm = np.r_[lo:hi, (0 if h else HALF):(HALF if h else D_INNER)]

    u = hs[b]
    if rev:
        u = u[::-1]
    uT = np.ascontiguousarray(u.T).astype(BF_NP)

    W_in_x = W_in[0:D_INNER][perm]
    W_in_z = W_in[D_INNER + lo:D_INNER + hi]
    conv_wp = conv_w[perm]
    conv_bp = conv_b[perm].reshape(-1, 1).astype(np.float32)
    W_xp = W_x[:, perm]
    W_xpad = np.zeros((NXP, W_xp.shape[1]), W_xp.dtype)
    W_xpad[0:DT_RANK] = W_xp[0:DT_RANK]
    W_xpad[64:96] = W_xp[DT_RANK:NXD]

    conv_diag = np.zeros((NDT_FULL * D_CONV * 128, 128), np.float32)
    idx = np.arange(128)
    for r in range(NDT_FULL):
        for j in range(D_CONV):
            base = (r * D_CONV + j) * 128
            conv_diag[base + idx, idx] = conv_wp[r * 128:(r + 1) * 128, j]

    dp_diag = np.zeros((NDT * 128, 128), np.float32)
    for r in range(NDT):
        dp_diag[r * 128 + idx, idx] = Dp[lo + r * 128: lo + (r + 1) * 128]

    return {
        "uT": uT,
        "w_in_xT": np.ascontiguousarray(W_in_x.T).astype(BF_NP),
        "w_in_zT": np.ascontiguousarray(W_in_z.T).astype(BF_NP),
        "conv_diag": conv_diag.astype(BF_NP),
        "conv_b": conv_bp,
        "w_xT": np.ascontiguousarray(W_xpad.T).astype(BF_NP),
        "w_dtT": np.ascontiguousarray(W_dt[lo:hi].T).astype(BF_NP),
        "b_dt": b_dt[lo:hi].reshape(-1, 1).astype(np.float32),
        "A_half": A[lo:hi].astype(np.float32),
        "dp_diag": dp_diag.astype(BF_NP),
        "idn": np.eye(128, dtype=np.float32).astype(BF_NP),
        "w_outT": np.ascontiguousarray(W_out[:, lo:hi].T).astype(BF_NP),
    }


_CACHE = {}


def kernel(**inputs):
    if "prog" not in _CACHE:
        _CACHE["prog"] = build_program(0)
    nc = _CACHE["prog"]

    in_maps = []
    for c in range(8):
        b, rev, h = c >> 2, (c >> 1) & 1, c & 1
        in_maps.append(_prep_core_inputs(inputs, b, rev, h))
    res = run_bass_kernel_spmd(nc, in_maps, list(range(8)))

    out = np.zeros((BATCH, L, D_MODEL), np.float32)
    for c in range(8):
        b, rev, h = c >> 2, (c >> 1) & 1, c & 1
        part = (res.results[c]["out_part"]
                + res.results[c]["out_part5"].astype(np.float32)).T
        if rev:
            part = part[::-1]
        out[b] += part
    return out

